# revision 5
# baseline (speedup 1.0000x reference)
"""AttnBlock (GroupNorm + single-head 1x1-conv attention + residual) on 8
Trainium2 NeuronCores.

Sharding: data-parallel over batch (4) x sequence-parallel over query tokens
(2 halves of 4096). Each core receives its batch element with the spatial
columns rotated so that its 2048 query tokens are always columns 0:2047 —
attention is invariant to key order, so one shared NEFF serves all cores.

Compute dtype: fp16 on the PE (full-rate), fp32 PSUM accumulation, fp32
softmax denominators and GroupNorm statistics.
"""

import numpy as np

P = 128
C = 512
KC = C // P          # 4 channel chunks of 128
N = 4096             # tokens (64*64)
NH = N // 2          # query tokens per core
G = 32               # groupnorm groups
GS = C // G          # 16 channels per group
EPS = 1e-6
N_CORES = 8

_CACHE = {}


def _apply_walrus_workarounds():
    """The walrus build in this container rejects any instruction carrying
    more than one semaphore wait ("Too many sync wait commands"). Split extra
    waits onto same-engine single-wait NOPs committed just before, and split
    the final TileContext drain the same way."""
    import concourse.tile as tile
    from concourse import mybir

    if getattr(tile.TileContext, "_walrus_wait_split", False):
        return

    _orig_commit = tile.TileContext._commit_instruction

    def _split_waits_commit(self, inst, lazy_reg_writes=True):
        si = inst.sync_info
        if si is not None and si.on_wait and len(si.on_wait) > 1 \
                and inst.engine != mybir.EngineType.Unassigned:
            waits = list(si.on_wait)
            si.on_wait = waits[-1:]
            for w in waits[:-1]:
                nop = mybir.InstNoOp(
                    name=self.nc.get_next_instruction_name(),
                    engine=inst.engine,
                    sync_info=mybir.SyncInfo(on_wait=[w], on_update=[]),
                    bass_nofuse=True,
                )
                _orig_commit(self, nop, lazy_reg_writes=False)
        return _orig_commit(self, inst, lazy_reg_writes=lazy_reg_writes)

    def _split_drain_and_barrier(self, tick_clock, wait_clock):
        nc = self.nc
        drain_inst = nc.sync.drain()
        wait_clock.add_sem_waits(
            drain_inst.ins, tile.ScopedClock({None: tick_clock.global_clock})
        )
        si = drain_inst.ins.sync_info
        waits = list(si.on_wait) if si is not None else []
        if len(waits) > 1:
            si.on_wait = waits[:1]
            for w in waits[1:]:
                d2 = nc.sync.drain()
                d2.ins.sync_info = mybir.SyncInfo(on_wait=[w], on_update=[])

        nc.all_engine_barrier()
        assert self.sems is not None
        popped = nc._tile_sem_poison_stack.pop()
        assert popped is self._sem_poison
        nc.clear_and_free_semaphores(list(self.sems.allocated().values()))
        nc.all_engine_barrier()

    tile.TileContext._commit_instruction = _split_waits_commit
    tile.TileContext._drain_and_barrier = _split_drain_and_barrier
    tile.TileContext._walrus_wait_split = True


def _build():
    """Trace the Bass/Tile program once; returns the Bass module."""
    import concourse.bass as bass
    import concourse.tile as tile
    from concourse import mybir

    _apply_walrus_workarounds()

    DT = mybir.dt.float16
    F32 = mybir.dt.float32

    nc = bass.Bass("TRN2", target_bir_lowering=False, debug=False, num_devices=1)

    xr = nc.dram_tensor("xr", [C, N], F32, kind="ExternalInput").ap()
    wq = nc.dram_tensor("wq", [C, C], DT, kind="ExternalInput").ap()
    wk = nc.dram_tensor("wk", [C, C], DT, kind="ExternalInput").ap()
    wv = nc.dram_tensor("wv", [C, C], DT, kind="ExternalInput").ap()
    wo = nc.dram_tensor("wo", [C, C], DT, kind="ExternalInput").ap()
    bq = nc.dram_tensor("bq", [C], F32, kind="ExternalInput").ap()
    bk = nc.dram_tensor("bk", [C], F32, kind="ExternalInput").ap()
    bo = nc.dram_tensor("bo", [C], F32, kind="ExternalInput").ap()
    gam = nc.dram_tensor("gam", [C], F32, kind="ExternalInput").ap()
    bet = nc.dram_tensor("bet", [C], F32, kind="ExternalInput").ap()
    gavg = nc.dram_tensor("gavg", [P, P], F32, kind="ExternalInput").ap()
    ident = nc.dram_tensor("ident", [P, P], DT, kind="ExternalInput").ap()
    y = nc.dram_tensor("y", [C, NH], F32, kind="ExternalOutput").ap()

    xr_t = xr.rearrange("(kc p) n -> kc p n", p=P)     # [4, 128, 4096]
    y_t = y.rearrange("(oc p) n -> oc p n", p=P)       # [4, 128, 2048]

    IB = NH // P        # 16 query blocks per core
    JQ = N // 512       # 8 key chunks of 512
    NHQ = NH // 512     # 4 query-token chunks of 512

    with tile.TileContext(nc) as tc:
        import contextlib
        ctx = contextlib.ExitStack()
        with ctx:
            consts = ctx.enter_context(tc.tile_pool(name="consts", bufs=1))
            xo_pool = ctx.enter_context(tc.tile_pool(name="xo", bufs=2))
            big = ctx.enter_context(tc.tile_pool(name="big", bufs=1))
            small = ctx.enter_context(tc.tile_pool(name="small", bufs=4))
            epool = ctx.enter_context(tc.tile_pool(name="epool", bufs=3))
            rpool = ctx.enter_context(tc.tile_pool(name="rpool", bufs=3))
            ps = ctx.enter_context(tc.tile_pool(name="ps", bufs=7, space="PSUM"))
            psg = ctx.enter_context(tc.tile_pool(name="psg", bufs=1, space="PSUM"))

            # ---- constants ------------------------------------------------
            w_sb = {}
            for name, ap in (("wq", wq), ("wk", wk), ("wv", wv), ("wo", wo)):
                t = consts.tile([P, KC, C], DT, tag=f"w_{name}")
                nc.sync.dma_start(t[:], ap.rearrange("(kc p) o -> p kc o", p=P))
                w_sb[name] = t
            b_sb = {}
            for name, ap in (("bq", bq), ("bk", bk), ("bo", bo),
                             ("gam", gam), ("bet", bet)):
                t = consts.tile([P, KC], F32, tag=f"b_{name}")
                nc.sync.dma_start(t[:], ap.rearrange("(kc p) -> p kc", p=P))
                b_sb[name] = t
            gavg_sb = consts.tile([P, P], F32, tag="gavg")
            nc.sync.dma_start(gavg_sb[:], gavg)
            ident_sb = consts.tile([P, P], DT, tag="ident")
            nc.sync.dma_start(ident_sb[:], ident)
            eps_sb = consts.tile([P, 1], F32, tag="eps")
            nc.vector.memset(eps_sb[:], EPS)

            # ---- phase 1: GroupNorm -> hn (fp16) --------------------------
            hn = big.tile([P, KC, N], DT, tag="hn")
            for kc in range(KC):
                x_c = xo_pool.tile([P, N], F32, tag="xo")
                nc.sync.dma_start(x_c[:], xr_t[kc])

                st = small.tile([P, 8, 6], F32, tag="bnst")
                xv = x_c.rearrange("p (s f) -> p s f", f=512)
                for s in range(8):
                    nc.vector.bn_stats(st[:, s, :], xv[:, s, :])
                mv = small.tile([P, 2], F32, tag="bnmv")
                nc.vector.bn_aggr(mv[:], st[:])

                # mv2 = [mean, E[x^2]] per partition
                mv2 = small.tile([P, 2], F32, tag="mv2")
                nc.vector.tensor_copy(mv2[:, 0:1], mv[:, 0:1])
                nc.vector.tensor_tensor(
                    mv2[:, 1:2], mv[:, 0:1], mv[:, 0:1], mybir.AluOpType.mult)
                nc.vector.tensor_tensor(
                    mv2[:, 1:2], mv2[:, 1:2], mv[:, 1:2], mybir.AluOpType.add)

                # group-average (and broadcast back to partitions) via PE
                g_ps = psg.tile([P, 2], F32, tag="gn")
                nc.tensor.matmul(g_ps[:], gavg_sb[:], mv2[:], start=True, stop=True)

                # var_g = E2_g - mean_g^2 ; rstd = 1/sqrt(var_g + eps)
                g_sb = small.tile([P, 2], F32, tag="gsb")
                nc.vector.tensor_copy(g_sb[:], g_ps[:])
                var_t = small.tile([P, 1], F32, tag="var")
                nc.vector.tensor_tensor(
                    var_t[:], g_sb[:, 0:1], g_sb[:, 0:1], mybir.AluOpType.mult)
                nc.vector.tensor_tensor(
                    var_t[:], g_sb[:, 1:2], var_t[:], mybir.AluOpType.subtract)
                sq = small.tile([P, 1], F32, tag="sq")
                nc.scalar.activation(
                    sq[:], var_t[:], mybir.ActivationFunctionType.Sqrt,
                    bias=eps_sb[:], scale=1.0)
                rstd = small.tile([P, 1], F32, tag="rstd")
                nc.vector.reciprocal(rstd[:], sq[:])

                # scale = rstd * gamma ; shift = beta - mean_g * scale
                scl = small.tile([P, 1], F32, tag="scl")
                nc.vector.tensor_tensor(
                    scl[:], rstd[:], b_sb["gam"][:, kc:kc + 1], mybir.AluOpType.mult)
                sh = small.tile([P, 1], F32, tag="sh")
                nc.vector.tensor_tensor(
                    sh[:], g_sb[:, 0:1], scl[:], mybir.AluOpType.mult)
                nc.vector.tensor_tensor(
                    sh[:], b_sb["bet"][:, kc:kc + 1], sh[:], mybir.AluOpType.subtract)

                nc.vector.tensor_scalar(
                    out=hn[:, kc, :], in0=x_c[:], scalar1=scl[:], scalar2=sh[:],
                    op0=mybir.AluOpType.mult, op1=mybir.AluOpType.add)

            # ---- phase 2: projections ------------------------------------
            k_sb = big.tile([P, KC, N], DT, tag="k")
            q_sb = big.tile([P, KC, NH], DT, tag="q")
            vt_sb = big.tile([P, N // P, C], DT, tag="vt")

            for oc in range(KC):
                for nt in range(JQ):
                    pp = ps.tile([P, 512], F32, tag="mm")
                    for kc in range(KC):
                        nc.tensor.matmul(
                            pp[:], w_sb["wk"][:, kc, oc * P:(oc + 1) * P],
                            hn[:, kc, nt * 512:(nt + 1) * 512],
                            start=(kc == 0), stop=(kc == KC - 1))
                    nc.scalar.activation(
                        k_sb[:, oc, nt * 512:(nt + 1) * 512], pp[:],
                        mybir.ActivationFunctionType.Identity,
                        bias=b_sb["bk"][:, oc:oc + 1], scale=1.0)
            for oc in range(KC):
                for nt in range(NHQ):
                    pp = ps.tile([P, 512], F32, tag="mm")
                    for kc in range(KC):
                        nc.tensor.matmul(
                            pp[:], w_sb["wq"][:, kc, oc * P:(oc + 1) * P],
                            hn[:, kc, nt * 512:(nt + 1) * 512],
                            start=(kc == 0), stop=(kc == KC - 1))
                    nc.scalar.activation(
                        q_sb[:, oc, nt * 512:(nt + 1) * 512], pp[:],
                        mybir.ActivationFunctionType.Identity,
                        bias=b_sb["bq"][:, oc:oc + 1], scale=1.0)
            for jc in range(N // P):
                pp = ps.tile([P, 512], F32, tag="mm")
                for kc in range(KC):
                    nc.tensor.matmul(
                        pp[:], hn[:, kc, jc * P:(jc + 1) * P], w_sb["wv"][:, kc, :],
                        start=(kc == 0), stop=(kc == KC - 1))
                nc.vector.tensor_copy(vt_sb[:, jc, :], pp[:])

            # ---- phase 3: attention, 16 query blocks ---------------------
            o_sb = xo_pool.tile([P, KC, NH], DT, tag="xo")
            for ib in range(IB):
                ssum = small.tile([P, JQ], F32, tag="ssum")
                ot_ps = ps.tile([P, C], F32, tag="mm")
                for jq in range(JQ):
                    s_ps = ps.tile([P, 512], F32, tag="mm")
                    for kc in range(KC):
                        nc.tensor.matmul(
                            s_ps[:], q_sb[:, kc, ib * P:(ib + 1) * P],
                            k_sb[:, kc, jq * 512:(jq + 1) * 512],
                            start=(kc == 0), stop=(kc == KC - 1))
                    e_sb = epool.tile([P, 512], DT, tag="e")
                    nc.scalar.activation(
                        e_sb[:], s_ps[:], mybir.ActivationFunctionType.Exp,
                        accum_out=ssum[:, jq:jq + 1])
                    t_ps = ps.tile([P, 512], DT, tag="mm")
                    for jj in range(4):
                        nc.tensor.transpose(
                            t_ps[:, jj * P:(jj + 1) * P],
                            e_sb[:, jj * P:(jj + 1) * P], ident_sb[:])
                    at_sb = epool.tile([P, 4, P], DT, tag="at")
                    nc.vector.tensor_copy(
                        at_sb[:], t_ps.rearrange("p (a b) -> p a b", b=P))
                    for jj in range(4):
                        nc.tensor.matmul(
                            ot_ps[:], at_sb[:, jj, :], vt_sb[:, jq * 4 + jj, :],
                            start=(jq == 0 and jj == 0),
                            stop=(jq == JQ - 1 and jj == 3))

                ssum_r = small.tile([P, 1], F32, tag="ssum_r")
                nc.vector.tensor_reduce(
                    ssum_r[:], ssum[:], mybir.AxisListType.X, mybir.AluOpType.add)
                recip = small.tile([P, 1], F32, tag="recip")
                nc.vector.reciprocal(recip[:], ssum_r[:])

                ot_sb = epool.tile([P, C], DT, tag="ot")
                nc.scalar.activation(
                    ot_sb[:], ot_ps[:], mybir.ActivationFunctionType.Copy,
                    scale=recip[:])
                to_ps = ps.tile([P, C], DT, tag="mm")
                for cb in range(KC):
                    nc.tensor.transpose(
                        to_ps[:, cb * P:(cb + 1) * P],
                        ot_sb[:, cb * P:(cb + 1) * P], ident_sb[:])
                nc.vector.tensor_copy(
                    o_sb[:, :, ib * P:(ib + 1) * P],
                    to_ps.rearrange("p (a b) -> p a b", b=P))

            # ---- phase 4: output projection + residual -------------------
            for oc in range(KC):
                for nt in range(NHQ):
                    pp = ps.tile([P, 512], F32, tag="mm")
                    for kc in range(KC):
                        nc.tensor.matmul(
                            pp[:], w_sb["wo"][:, kc, oc * P:(oc + 1) * P],
                            o_sb[:, kc, nt * 512:(nt + 1) * 512],
                            start=(kc == 0), stop=(kc == KC - 1))
                    x_res = rpool.tile([P, 512], F32, tag="xres")
                    nc.sync.dma_start(
                        x_res[:], xr_t[oc][:, nt * 512:(nt + 1) * 512])
                    r_sb = rpool.tile([P, 512], F32, tag="r")
                    nc.scalar.activation(
                        r_sb[:], pp[:], mybir.ActivationFunctionType.Identity,
                        bias=b_sb["bo"][:, oc:oc + 1], scale=1.0)
                    nc.vector.tensor_tensor(
                        r_sb[:], r_sb[:], x_res[:], mybir.AluOpType.add)
                    nc.sync.dma_start(y_t[oc][:, nt * 512:(nt + 1) * 512], r_sb[:])

    return nc


def _prep_in_maps(inputs):
    x = np.asarray(inputs["x"], np.float32).reshape(4, C, N)
    s = np.float32(C ** -0.5)
    wq = np.asarray(inputs["wq"], np.float32)
    wk = np.asarray(inputs["wk"], np.float32)
    wv = np.asarray(inputs["wv"], np.float32)
    wo = np.asarray(inputs["wo"], np.float32)
    shared = {
        "wq": np.ascontiguousarray((wq * s).T).astype(np.float16),
        "wk": np.ascontiguousarray(wk.T).astype(np.float16),
        "wv": np.ascontiguousarray(wv.T).astype(np.float16),
        "wo": np.ascontiguousarray(wo.T).astype(np.float16),
        "bq": (np.asarray(inputs["bq"], np.float32) * s).astype(np.float32),
        "bk": np.asarray(inputs["bk"], np.float32),
        "bo": (wo @ np.asarray(inputs["bv"], np.float32)
               + np.asarray(inputs["bo"], np.float32)).astype(np.float32),
        "gam": np.asarray(inputs["gamma"], np.float32),
        "bet": np.asarray(inputs["beta"], np.float32),
        "gavg": (np.kron(np.eye(P // GS, dtype=np.float32),
                         np.ones((GS, GS), np.float32)) / GS),
        "ident": np.eye(P, dtype=np.float16),
    }
    in_maps = []
    for core in range(N_CORES):
        b, half = divmod(core, 2)
        xb = x[b]
        if half == 1:
            xrot = np.ascontiguousarray(
                np.concatenate([xb[:, NH:], xb[:, :NH]], axis=1))
        else:
            xrot = np.ascontiguousarray(xb)
        in_maps.append({"xr": xrot, **shared})
    return in_maps


def kernel_run(inputs, trace=False, trace_cores=None):
    """Run on all 8 cores; returns (full_output, BassKernelResults)."""
    from concourse.bass_utils import run_bass_kernel_spmd

    if "nc" not in _CACHE:
        _CACHE["nc"] = _build()
    nc = _CACHE["nc"]
    in_maps = _prep_in_maps(inputs)
    res = run_bass_kernel_spmd(
        nc, in_maps, core_ids=list(range(N_CORES)), trace=trace,
        trace_cores=trace_cores)
    out = np.empty((4, C, N), np.float32)
    for core in range(N_CORES):
        b, half = divmod(core, 2)
        out[b][:, half * NH:(half + 1) * NH] = res.results[core]["y"]
    return out.reshape(4, C, 64, 64), res


def kernel(**inputs):
    out, _ = kernel_run(inputs, trace=False)
    return out


# revision 7
# speedup vs baseline: 1.4558x; 1.4558x over previous
"""AttnBlock (GroupNorm + single-head 1x1-conv attention + residual) on 8
Trainium2 NeuronCores.

Sharding: data-parallel over batch (4) x sequence-parallel over query tokens
(2 halves of 4096). Each core receives its batch element with the spatial
columns rotated so that its 2048 query tokens are always columns 0:2047 —
attention is invariant to key order, so one shared NEFF serves all cores.

Compute dtype: fp16 on the PE (full-rate), fp32 PSUM accumulation, fp32
softmax denominators and GroupNorm statistics.
"""

import numpy as np

P = 128
C = 512
KC = C // P          # 4 channel chunks of 128
N = 4096             # tokens (64*64)
NH = N // 2          # query tokens per core
G = 32               # groupnorm groups
GS = C // G          # 16 channels per group
EPS = 1e-6
N_CORES = 8

_CACHE = {}


def _apply_walrus_workarounds():
    """The walrus build in this container rejects any instruction carrying
    more than one semaphore wait ("Too many sync wait commands"). Split extra
    waits onto same-engine single-wait NOPs committed just before, and split
    the final TileContext drain the same way."""
    import concourse.tile as tile
    from concourse import mybir

    if getattr(tile.TileContext, "_walrus_wait_split", False):
        return

    _orig_commit = tile.TileContext._commit_instruction

    def _split_waits_commit(self, inst, lazy_reg_writes=True):
        si = inst.sync_info
        if si is not None and si.on_wait and len(si.on_wait) > 1 \
                and inst.engine != mybir.EngineType.Unassigned:
            waits = list(si.on_wait)
            si.on_wait = waits[-1:]
            for w in waits[:-1]:
                nop = mybir.InstNoOp(
                    name=self.nc.get_next_instruction_name(),
                    engine=inst.engine,
                    sync_info=mybir.SyncInfo(on_wait=[w], on_update=[]),
                    bass_nofuse=True,
                )
                _orig_commit(self, nop, lazy_reg_writes=False)
        return _orig_commit(self, inst, lazy_reg_writes=lazy_reg_writes)

    def _split_drain_and_barrier(self, tick_clock, wait_clock):
        nc = self.nc
        drain_inst = nc.sync.drain()
        wait_clock.add_sem_waits(
            drain_inst.ins, tile.ScopedClock({None: tick_clock.global_clock})
        )
        si = drain_inst.ins.sync_info
        waits = list(si.on_wait) if si is not None else []
        if len(waits) > 1:
            si.on_wait = waits[:1]
            for w in waits[1:]:
                d2 = nc.sync.drain()
                d2.ins.sync_info = mybir.SyncInfo(on_wait=[w], on_update=[])

        nc.all_engine_barrier()
        assert self.sems is not None
        popped = nc._tile_sem_poison_stack.pop()
        assert popped is self._sem_poison
        nc.clear_and_free_semaphores(list(self.sems.allocated().values()))
        nc.all_engine_barrier()

    tile.TileContext._commit_instruction = _split_waits_commit
    tile.TileContext._drain_and_barrier = _split_drain_and_barrier
    tile.TileContext._walrus_wait_split = True


def _build():
    """Trace the Bass/Tile program once; returns the Bass module."""
    import concourse.bass as bass
    import concourse.tile as tile
    from concourse import mybir

    _apply_walrus_workarounds()

    DT = mybir.dt.float16
    F32 = mybir.dt.float32

    nc = bass.Bass("TRN2", target_bir_lowering=False, debug=False, num_devices=1)

    xr = nc.dram_tensor("xr", [C, N], F32, kind="ExternalInput").ap()
    wq = nc.dram_tensor("wq", [C, C], DT, kind="ExternalInput").ap()
    wk = nc.dram_tensor("wk", [C, C], DT, kind="ExternalInput").ap()
    wv = nc.dram_tensor("wv", [C, C], DT, kind="ExternalInput").ap()
    wo = nc.dram_tensor("wo", [C, C], DT, kind="ExternalInput").ap()
    bq = nc.dram_tensor("bq", [C], F32, kind="ExternalInput").ap()
    bk = nc.dram_tensor("bk", [C], F32, kind="ExternalInput").ap()
    bo = nc.dram_tensor("bo", [C], F32, kind="ExternalInput").ap()
    gam = nc.dram_tensor("gam", [C], F32, kind="ExternalInput").ap()
    bet = nc.dram_tensor("bet", [C], F32, kind="ExternalInput").ap()
    gavg = nc.dram_tensor("gavg", [P, P], F32, kind="ExternalInput").ap()
    ident = nc.dram_tensor("ident", [P, P], DT, kind="ExternalInput").ap()
    y = nc.dram_tensor("y", [C, NH], F32, kind="ExternalOutput").ap()

    xr_t = xr.rearrange("(kc p) n -> kc p n", p=P)     # [4, 128, 4096]
    y_t = y.rearrange("(oc p) n -> oc p n", p=P)       # [4, 128, 2048]

    IB = NH // P        # 16 query blocks per core
    JQ = N // 512       # 8 key chunks of 512
    NHQ = NH // 512     # 4 query-token chunks of 512

    with tile.TileContext(nc) as tc:
        import contextlib
        ctx = contextlib.ExitStack()
        with ctx:
            consts = ctx.enter_context(tc.tile_pool(name="consts", bufs=1))
            xo_pool = ctx.enter_context(tc.tile_pool(name="xo", bufs=2))
            big = ctx.enter_context(tc.tile_pool(name="big", bufs=1))
            small = ctx.enter_context(tc.tile_pool(name="small", bufs=4))
            epool = ctx.enter_context(tc.tile_pool(name="epool", bufs=3))
            rpool = ctx.enter_context(tc.tile_pool(name="rpool", bufs=3))
            ps = ctx.enter_context(tc.tile_pool(name="ps", bufs=7, space="PSUM"))
            psg = ctx.enter_context(tc.tile_pool(name="psg", bufs=1, space="PSUM"))

            # ---- constants ------------------------------------------------
            w_sb = {}
            for name, ap in (("wq", wq), ("wk", wk), ("wv", wv), ("wo", wo)):
                t = consts.tile([P, KC, C], DT, tag=f"w_{name}")
                nc.sync.dma_start(t[:], ap.rearrange("(kc p) o -> p kc o", p=P))
                w_sb[name] = t
            b_sb = {}
            for name, ap in (("bq", bq), ("bk", bk), ("bo", bo),
                             ("gam", gam), ("bet", bet)):
                t = consts.tile([P, KC], F32, tag=f"b_{name}")
                nc.sync.dma_start(t[:], ap.rearrange("(kc p) -> p kc", p=P))
                b_sb[name] = t
            gavg_sb = consts.tile([P, P], F32, tag="gavg")
            nc.sync.dma_start(gavg_sb[:], gavg)
            ident_sb = consts.tile([P, P], DT, tag="ident")
            nc.sync.dma_start(ident_sb[:], ident)
            eps_sb = consts.tile([P, 1], F32, tag="eps")
            nc.vector.memset(eps_sb[:], EPS)

            # ---- phase 1: GroupNorm -> hn (fp16) --------------------------
            hn = big.tile([P, KC, N], DT, tag="hn")
            for kc in range(KC):
                x_c = xo_pool.tile([P, N], F32, tag="xo")
                nc.sync.dma_start(x_c[:], xr_t[kc])

                st = small.tile([P, 8, 6], F32, tag="bnst")
                xv = x_c.rearrange("p (s f) -> p s f", f=512)
                for s in range(8):
                    nc.vector.bn_stats(st[:, s, :], xv[:, s, :])
                mv = small.tile([P, 2], F32, tag="bnmv")
                nc.vector.bn_aggr(mv[:], st[:])

                # mv2 = [mean, E[x^2]] per partition
                mv2 = small.tile([P, 2], F32, tag="mv2")
                nc.vector.tensor_copy(mv2[:, 0:1], mv[:, 0:1])
                nc.vector.tensor_tensor(
                    mv2[:, 1:2], mv[:, 0:1], mv[:, 0:1], mybir.AluOpType.mult)
                nc.vector.tensor_tensor(
                    mv2[:, 1:2], mv2[:, 1:2], mv[:, 1:2], mybir.AluOpType.add)

                # group-average (and broadcast back to partitions) via PE
                g_ps = psg.tile([P, 2], F32, tag="gn")
                nc.tensor.matmul(g_ps[:], gavg_sb[:], mv2[:], start=True, stop=True)

                # var_g = E2_g - mean_g^2 ; rstd = 1/sqrt(var_g + eps)
                g_sb = small.tile([P, 2], F32, tag="gsb")
                nc.vector.tensor_copy(g_sb[:], g_ps[:])
                var_t = small.tile([P, 1], F32, tag="var")
                nc.vector.tensor_tensor(
                    var_t[:], g_sb[:, 0:1], g_sb[:, 0:1], mybir.AluOpType.mult)
                nc.vector.tensor_tensor(
                    var_t[:], g_sb[:, 1:2], var_t[:], mybir.AluOpType.subtract)
                sq = small.tile([P, 1], F32, tag="sq")
                nc.scalar.activation(
                    sq[:], var_t[:], mybir.ActivationFunctionType.Sqrt,
                    bias=eps_sb[:], scale=1.0)
                rstd = small.tile([P, 1], F32, tag="rstd")
                nc.vector.reciprocal(rstd[:], sq[:])

                # scale = rstd * gamma ; shift = beta - mean_g * scale
                scl = small.tile([P, 1], F32, tag="scl")
                nc.vector.tensor_tensor(
                    scl[:], rstd[:], b_sb["gam"][:, kc:kc + 1], mybir.AluOpType.mult)
                sh = small.tile([P, 1], F32, tag="sh")
                nc.vector.tensor_tensor(
                    sh[:], g_sb[:, 0:1], scl[:], mybir.AluOpType.mult)
                nc.vector.tensor_tensor(
                    sh[:], b_sb["bet"][:, kc:kc + 1], sh[:], mybir.AluOpType.subtract)

                nc.vector.tensor_scalar(
                    out=hn[:, kc, :], in0=x_c[:], scalar1=scl[:], scalar2=sh[:],
                    op0=mybir.AluOpType.mult, op1=mybir.AluOpType.add)

            # ---- phase 2: projections ------------------------------------
            k_sb = big.tile([P, KC, N], DT, tag="k")
            q_sb = big.tile([P, KC, NH], DT, tag="q")
            vt_sb = big.tile([P, N // P, C], DT, tag="vt")

            for oc in range(KC):
                for nt in range(JQ):
                    pp = ps.tile([P, 512], F32, tag="mm")
                    for kc in range(KC):
                        nc.tensor.matmul(
                            pp[:], w_sb["wk"][:, kc, oc * P:(oc + 1) * P],
                            hn[:, kc, nt * 512:(nt + 1) * 512],
                            start=(kc == 0), stop=(kc == KC - 1))
                    nc.scalar.activation(
                        k_sb[:, oc, nt * 512:(nt + 1) * 512], pp[:],
                        mybir.ActivationFunctionType.Identity,
                        bias=b_sb["bk"][:, oc:oc + 1], scale=1.0)
            for oc in range(KC):
                for nt in range(NHQ):
                    pp = ps.tile([P, 512], F32, tag="mm")
                    for kc in range(KC):
                        nc.tensor.matmul(
                            pp[:], w_sb["wq"][:, kc, oc * P:(oc + 1) * P],
                            hn[:, kc, nt * 512:(nt + 1) * 512],
                            start=(kc == 0), stop=(kc == KC - 1))
                    nc.scalar.activation(
                        q_sb[:, oc, nt * 512:(nt + 1) * 512], pp[:],
                        mybir.ActivationFunctionType.Identity,
                        bias=b_sb["bq"][:, oc:oc + 1], scale=1.0)
            for jc in range(N // P):
                pp = ps.tile([P, 512], F32, tag="mm")
                for kc in range(KC):
                    nc.tensor.matmul(
                        pp[:], hn[:, kc, jc * P:(jc + 1) * P], w_sb["wv"][:, kc, :],
                        start=(kc == 0), stop=(kc == KC - 1))
                nc.vector.tensor_copy(vt_sb[:, jc, :], pp[:])

            # ---- phase 3: attention, 16 query blocks ---------------------
            # Flat software pipeline over global key chunks u = ib*JQ + jq:
            #   iter t:  S-matmuls + exp of chunk t
            #            transposes + AT copy + O^T matmuls of chunk t-1
            #            epilogue of block (t-2)//JQ when t-2 ends a block
            # so the PE never sits on the exp (ACT) latency of its own chunk.
            o_sb = xo_pool.tile([P, KC, NH], DT, tag="xo")
            TOT = IB * JQ
            e_hold = {}
            ssum_hold = {}
            ot_hold = {}

            def stage_s(u):
                ib, jq = divmod(u, JQ)
                if jq == 0:
                    ssum_hold[ib] = small.tile([P, JQ, 2], F32, tag="ssum", name=f"ssum{ib}")
                s_ps = ps.tile([P, 512], F32, tag="mm")
                for kc in range(KC):
                    nc.tensor.matmul(
                        s_ps[:], q_sb[:, kc, ib * P:(ib + 1) * P],
                        k_sb[:, kc, jq * 512:(jq + 1) * 512],
                        start=(kc == 0), stop=(kc == KC - 1))
                e_sb = epool.tile([P, 512], DT, tag="e")
                for h in range(2):
                    nc.scalar.activation(
                        e_sb[:, h * 256:(h + 1) * 256],
                        s_ps[:, h * 256:(h + 1) * 256],
                        mybir.ActivationFunctionType.Exp,
                        accum_out=ssum_hold[ib][:, jq, h:h + 1])
                e_hold[u] = e_sb

            def stage_to(u):
                ib, jq = divmod(u, JQ)
                if jq == 0:
                    ot_hold[ib] = ps.tile([P, C], F32, tag="mm", name=f"ot{ib}")
                ot_ps = ot_hold[ib]
                e_sb = e_hold.pop(u)
                t_ps = ps.tile([P, 512], DT, tag="mm")
                for jj in range(4):
                    nc.tensor.transpose(
                        t_ps[:, jj * P:(jj + 1) * P],
                        e_sb[:, jj * P:(jj + 1) * P], ident_sb[:])
                at_sb = epool.tile([P, 4, P], DT, tag="at")
                tv = t_ps.rearrange("p (a b) -> p a b", b=P)
                nc.vector.tensor_copy(at_sb[:, 0:2], tv[:, 0:2])
                nc.vector.tensor_copy(at_sb[:, 2:4], tv[:, 2:4])
                for jj in range(4):
                    nc.tensor.matmul(
                        ot_ps[:], at_sb[:, jj, :], vt_sb[:, jq * 4 + jj, :],
                        start=(jq == 0 and jj == 0),
                        stop=(jq == JQ - 1 and jj == 3))

            def stage_epi(ib):
                ssum = ssum_hold.pop(ib)
                ot_ps = ot_hold.pop(ib)
                ssum_r = small.tile([P, 1], F32, tag="ssum_r")
                nc.vector.tensor_reduce(
                    ssum_r[:], ssum[:], mybir.AxisListType.XY, mybir.AluOpType.add)
                recip = small.tile([P, 1], F32, tag="recip")
                nc.vector.reciprocal(recip[:], ssum_r[:])

                ot_sb = epool.tile([P, C], DT, tag="ot")
                nc.scalar.activation(
                    ot_sb[:], ot_ps[:], mybir.ActivationFunctionType.Copy,
                    scale=recip[:])
                to_ps = ps.tile([P, C], DT, tag="mm")
                for cb in range(KC):
                    nc.tensor.transpose(
                        to_ps[:, cb * P:(cb + 1) * P],
                        ot_sb[:, cb * P:(cb + 1) * P], ident_sb[:])
                nc.vector.tensor_copy(
                    o_sb[:, :, ib * P:(ib + 1) * P],
                    to_ps.rearrange("p (a b) -> p a b", b=P))

            for t in range(TOT + 2):
                if t < TOT:
                    stage_s(t)
                if 1 <= t <= TOT:
                    stage_to(t - 1)
                if t >= 2 and (t - 2) % JQ == JQ - 1:
                    stage_epi((t - 2) // JQ)

            # ---- phase 4: output projection + residual -------------------
            for oc in range(KC):
                for nt in range(NHQ):
                    pp = ps.tile([P, 512], F32, tag="mm")
                    for kc in range(KC):
                        nc.tensor.matmul(
                            pp[:], w_sb["wo"][:, kc, oc * P:(oc + 1) * P],
                            o_sb[:, kc, nt * 512:(nt + 1) * 512],
                            start=(kc == 0), stop=(kc == KC - 1))
                    x_res = rpool.tile([P, 512], F32, tag="xres")
                    nc.sync.dma_start(
                        x_res[:], xr_t[oc][:, nt * 512:(nt + 1) * 512])
                    r_sb = rpool.tile([P, 512], F32, tag="r")
                    nc.scalar.activation(
                        r_sb[:], pp[:], mybir.ActivationFunctionType.Identity,
                        bias=b_sb["bo"][:, oc:oc + 1], scale=1.0)
                    nc.vector.tensor_tensor(
                        r_sb[:], r_sb[:], x_res[:], mybir.AluOpType.add)
                    nc.sync.dma_start(y_t[oc][:, nt * 512:(nt + 1) * 512], r_sb[:])

    return nc


def _prep_in_maps(inputs):
    x = np.asarray(inputs["x"], np.float32).reshape(4, C, N)
    s = np.float32(C ** -0.5)
    wq = np.asarray(inputs["wq"], np.float32)
    wk = np.asarray(inputs["wk"], np.float32)
    wv = np.asarray(inputs["wv"], np.float32)
    wo = np.asarray(inputs["wo"], np.float32)
    shared = {
        "wq": np.ascontiguousarray((wq * s).T).astype(np.float16),
        "wk": np.ascontiguousarray(wk.T).astype(np.float16),
        "wv": np.ascontiguousarray(wv.T).astype(np.float16),
        "wo": np.ascontiguousarray(wo.T).astype(np.float16),
        "bq": (np.asarray(inputs["bq"], np.float32) * s).astype(np.float32),
        "bk": np.asarray(inputs["bk"], np.float32),
        "bo": (wo @ np.asarray(inputs["bv"], np.float32)
               + np.asarray(inputs["bo"], np.float32)).astype(np.float32),
        "gam": np.asarray(inputs["gamma"], np.float32),
        "bet": np.asarray(inputs["beta"], np.float32),
        "gavg": (np.kron(np.eye(P // GS, dtype=np.float32),
                         np.ones((GS, GS), np.float32)) / GS),
        "ident": np.eye(P, dtype=np.float16),
    }
    in_maps = []
    for core in range(N_CORES):
        b, half = divmod(core, 2)
        xb = x[b]
        if half == 1:
            xrot = np.ascontiguousarray(
                np.concatenate([xb[:, NH:], xb[:, :NH]], axis=1))
        else:
            xrot = np.ascontiguousarray(xb)
        in_maps.append({"xr": xrot, **shared})
    return in_maps


def kernel_run(inputs, trace=False, trace_cores=None):
    """Run on all 8 cores; returns (full_output, BassKernelResults)."""
    from concourse.bass_utils import run_bass_kernel_spmd

    if "nc" not in _CACHE:
        _CACHE["nc"] = _build()
    nc = _CACHE["nc"]
    in_maps = _prep_in_maps(inputs)
    res = run_bass_kernel_spmd(
        nc, in_maps, core_ids=list(range(N_CORES)), trace=trace,
        trace_cores=trace_cores)
    out = np.empty((4, C, N), np.float32)
    for core in range(N_CORES):
        b, half = divmod(core, 2)
        out[b][:, half * NH:(half + 1) * NH] = res.results[core]["y"]
    return out.reshape(4, C, 64, 64), res


def kernel(**inputs):
    out, _ = kernel_run(inputs, trace=False)
    return out


# revision 10
# speedup vs baseline: 1.5336x; 1.0534x over previous
"""AttnBlock (GroupNorm + single-head 1x1-conv attention + residual) on 8
Trainium2 NeuronCores.

Sharding: data-parallel over batch (4) x sequence-parallel over query tokens
(2 halves of 4096). Each core receives its batch element with the spatial
columns rotated so that its 2048 query tokens are always columns 0:2047 —
attention is invariant to key order, so one shared NEFF serves all cores.

Compute dtype: fp16 on the PE (full-rate), fp32 PSUM accumulation, fp32
softmax denominators and GroupNorm statistics.
"""

import numpy as np

P = 128
C = 512
KC = C // P          # 4 channel chunks of 128
N = 4096             # tokens (64*64)
NH = N // 2          # query tokens per core
G = 32               # groupnorm groups
GS = C // G          # 16 channels per group
EPS = 1e-6
N_CORES = 8

_CACHE = {}


def _apply_walrus_workarounds():
    """The walrus build in this container rejects any instruction carrying
    more than one semaphore wait ("Too many sync wait commands"). Split extra
    waits onto same-engine single-wait NOPs committed just before, and split
    the final TileContext drain the same way."""
    import concourse.tile as tile
    from concourse import mybir

    if getattr(tile.TileContext, "_walrus_wait_split", False):
        return

    _orig_commit = tile.TileContext._commit_instruction

    def _split_waits_commit(self, inst, lazy_reg_writes=True):
        si = inst.sync_info
        if si is not None and si.on_wait and len(si.on_wait) > 1 \
                and inst.engine != mybir.EngineType.Unassigned:
            waits = list(si.on_wait)
            si.on_wait = waits[-1:]
            for w in waits[:-1]:
                nop = mybir.InstNoOp(
                    name=self.nc.get_next_instruction_name(),
                    engine=inst.engine,
                    sync_info=mybir.SyncInfo(on_wait=[w], on_update=[]),
                    bass_nofuse=True,
                )
                _orig_commit(self, nop, lazy_reg_writes=False)
        return _orig_commit(self, inst, lazy_reg_writes=lazy_reg_writes)

    def _split_drain_and_barrier(self, tick_clock, wait_clock):
        nc = self.nc
        drain_inst = nc.sync.drain()
        wait_clock.add_sem_waits(
            drain_inst.ins, tile.ScopedClock({None: tick_clock.global_clock})
        )
        si = drain_inst.ins.sync_info
        waits = list(si.on_wait) if si is not None else []
        if len(waits) > 1:
            si.on_wait = waits[:1]
            for w in waits[1:]:
                d2 = nc.sync.drain()
                d2.ins.sync_info = mybir.SyncInfo(on_wait=[w], on_update=[])

        nc.all_engine_barrier()
        assert self.sems is not None
        popped = nc._tile_sem_poison_stack.pop()
        assert popped is self._sem_poison
        nc.clear_and_free_semaphores(list(self.sems.allocated().values()))
        nc.all_engine_barrier()

    tile.TileContext._commit_instruction = _split_waits_commit
    tile.TileContext._drain_and_barrier = _split_drain_and_barrier
    tile.TileContext._walrus_wait_split = True


def _build():
    """Trace the Bass/Tile program once; returns the Bass module."""
    import concourse.bass as bass
    import concourse.tile as tile
    from concourse import mybir

    _apply_walrus_workarounds()

    DT = mybir.dt.float16
    F32 = mybir.dt.float32

    nc = bass.Bass("TRN2", target_bir_lowering=False, debug=False, num_devices=1)

    xr = nc.dram_tensor("xr", [C, N], F32, kind="ExternalInput").ap()
    wq = nc.dram_tensor("wq", [C, C], DT, kind="ExternalInput").ap()
    wk = nc.dram_tensor("wk", [C, C], DT, kind="ExternalInput").ap()
    wv = nc.dram_tensor("wv", [C, C], DT, kind="ExternalInput").ap()
    wo = nc.dram_tensor("wo", [C, C], DT, kind="ExternalInput").ap()
    bq = nc.dram_tensor("bq", [C], F32, kind="ExternalInput").ap()
    bk = nc.dram_tensor("bk", [C], F32, kind="ExternalInput").ap()
    bo = nc.dram_tensor("bo", [C], F32, kind="ExternalInput").ap()
    gam = nc.dram_tensor("gam", [C], F32, kind="ExternalInput").ap()
    bet = nc.dram_tensor("bet", [C], F32, kind="ExternalInput").ap()
    gavg = nc.dram_tensor("gavg", [P, P], F32, kind="ExternalInput").ap()
    ident = nc.dram_tensor("ident", [P, P], DT, kind="ExternalInput").ap()
    y = nc.dram_tensor("y", [C, NH], F32, kind="ExternalOutput").ap()

    xr_t = xr.rearrange("(kc p) n -> kc p n", p=P)     # [4, 128, 4096]
    y_t = y.rearrange("(oc p) n -> oc p n", p=P)       # [4, 128, 2048]

    IB = NH // P        # 16 query blocks per core
    JQ = N // 512       # 8 key chunks of 512
    NHQ = NH // 512     # 4 query-token chunks of 512

    with tile.TileContext(nc) as tc:
        import contextlib
        ctx = contextlib.ExitStack()
        with ctx:
            consts = ctx.enter_context(tc.tile_pool(name="consts", bufs=1))
            xo_pool = ctx.enter_context(tc.tile_pool(name="xo", bufs=2))
            big = ctx.enter_context(tc.tile_pool(name="big", bufs=1))
            small = ctx.enter_context(tc.tile_pool(name="small", bufs=4))
            epool = ctx.enter_context(tc.tile_pool(name="epool", bufs=3))
            rpool = ctx.enter_context(tc.tile_pool(name="rpool", bufs=3))
            ps = ctx.enter_context(tc.tile_pool(name="ps", bufs=7, space="PSUM"))
            psg = ctx.enter_context(tc.tile_pool(name="psg", bufs=1, space="PSUM"))

            # ---- constants ------------------------------------------------
            w_sb = {}
            for name, ap in (("wq", wq), ("wk", wk), ("wv", wv), ("wo", wo)):
                t = consts.tile([P, KC, C], DT, tag=f"w_{name}")
                nc.sync.dma_start(t[:], ap.rearrange("(kc p) o -> p kc o", p=P))
                w_sb[name] = t
            b_sb = {}
            for name, ap in (("bq", bq), ("bk", bk), ("bo", bo),
                             ("gam", gam), ("bet", bet)):
                t = consts.tile([P, KC], F32, tag=f"b_{name}")
                nc.sync.dma_start(t[:], ap.rearrange("(kc p) -> p kc", p=P))
                b_sb[name] = t
            gavg_sb = consts.tile([P, P], F32, tag="gavg")
            nc.sync.dma_start(gavg_sb[:], gavg)
            ident_sb = consts.tile([P, P], DT, tag="ident")
            nc.sync.dma_start(ident_sb[:], ident)
            eps_sb = consts.tile([P, 1], F32, tag="eps")
            nc.vector.memset(eps_sb[:], EPS)

            # ---- phase 1: GroupNorm -> hn (fp16) --------------------------
            hn = big.tile([P, KC, N], DT, tag="hn")
            for kc in range(KC):
                x_c = xo_pool.tile([P, N], F32, tag="xo")
                nc.sync.dma_start(x_c[:], xr_t[kc])

                st = small.tile([P, 8, 6], F32, tag="bnst")
                xv = x_c.rearrange("p (s f) -> p s f", f=512)
                for s in range(8):
                    nc.vector.bn_stats(st[:, s, :], xv[:, s, :])
                mv = small.tile([P, 2], F32, tag="bnmv")
                nc.vector.bn_aggr(mv[:], st[:])

                # mv2 = [mean, E[x^2]] per partition
                mv2 = small.tile([P, 2], F32, tag="mv2")
                nc.vector.tensor_copy(mv2[:, 0:1], mv[:, 0:1])
                nc.vector.tensor_tensor(
                    mv2[:, 1:2], mv[:, 0:1], mv[:, 0:1], mybir.AluOpType.mult)
                nc.vector.tensor_tensor(
                    mv2[:, 1:2], mv2[:, 1:2], mv[:, 1:2], mybir.AluOpType.add)

                # group-average (and broadcast back to partitions) via PE
                g_ps = psg.tile([P, 2], F32, tag="gn")
                nc.tensor.matmul(g_ps[:], gavg_sb[:], mv2[:], start=True, stop=True)

                # var_g = E2_g - mean_g^2 ; rstd = 1/sqrt(var_g + eps)
                g_sb = small.tile([P, 2], F32, tag="gsb")
                nc.vector.tensor_copy(g_sb[:], g_ps[:])
                var_t = small.tile([P, 1], F32, tag="var")
                nc.vector.tensor_tensor(
                    var_t[:], g_sb[:, 0:1], g_sb[:, 0:1], mybir.AluOpType.mult)
                nc.vector.tensor_tensor(
                    var_t[:], g_sb[:, 1:2], var_t[:], mybir.AluOpType.subtract)
                sq = small.tile([P, 1], F32, tag="sq")
                nc.scalar.activation(
                    sq[:], var_t[:], mybir.ActivationFunctionType.Sqrt,
                    bias=eps_sb[:], scale=1.0)
                rstd = small.tile([P, 1], F32, tag="rstd")
                nc.vector.reciprocal(rstd[:], sq[:])

                # scale = rstd * gamma ; shift = beta - mean_g * scale
                scl = small.tile([P, 1], F32, tag="scl")
                nc.vector.tensor_tensor(
                    scl[:], rstd[:], b_sb["gam"][:, kc:kc + 1], mybir.AluOpType.mult)
                sh = small.tile([P, 1], F32, tag="sh")
                nc.vector.tensor_tensor(
                    sh[:], g_sb[:, 0:1], scl[:], mybir.AluOpType.mult)
                nc.vector.tensor_tensor(
                    sh[:], b_sb["bet"][:, kc:kc + 1], sh[:], mybir.AluOpType.subtract)

                nc.vector.tensor_scalar(
                    out=hn[:, kc, :], in0=x_c[:], scalar1=scl[:], scalar2=sh[:],
                    op0=mybir.AluOpType.mult, op1=mybir.AluOpType.add)

            # ---- phase 2: projections ------------------------------------
            k_sb = big.tile([P, KC, N], DT, tag="k")
            q_sb = big.tile([P, KC, NH], DT, tag="q")
            vt_sb = big.tile([P, N // P, C], DT, tag="vt")

            for oc in range(KC):
                for nt in range(JQ):
                    pp = ps.tile([P, 512], F32, tag="mm")
                    for kc in range(KC):
                        nc.tensor.matmul(
                            pp[:], w_sb["wk"][:, kc, oc * P:(oc + 1) * P],
                            hn[:, kc, nt * 512:(nt + 1) * 512],
                            start=(kc == 0), stop=(kc == KC - 1))
                    nc.scalar.activation(
                        k_sb[:, oc, nt * 512:(nt + 1) * 512], pp[:],
                        mybir.ActivationFunctionType.Identity,
                        bias=b_sb["bk"][:, oc:oc + 1], scale=1.0)
            for oc in range(KC):
                for nt in range(NHQ):
                    pp = ps.tile([P, 512], F32, tag="mm")
                    for kc in range(KC):
                        nc.tensor.matmul(
                            pp[:], w_sb["wq"][:, kc, oc * P:(oc + 1) * P],
                            hn[:, kc, nt * 512:(nt + 1) * 512],
                            start=(kc == 0), stop=(kc == KC - 1))
                    nc.scalar.activation(
                        q_sb[:, oc, nt * 512:(nt + 1) * 512], pp[:],
                        mybir.ActivationFunctionType.Identity,
                        bias=b_sb["bq"][:, oc:oc + 1], scale=1.0)
            for jc in range(N // P):
                pp = ps.tile([P, 512], F32, tag="mm")
                for kc in range(KC):
                    nc.tensor.matmul(
                        pp[:], hn[:, kc, jc * P:(jc + 1) * P], w_sb["wv"][:, kc, :],
                        start=(kc == 0), stop=(kc == KC - 1))
                nc.vector.tensor_copy(vt_sb[:, jc, :], pp[:])

            # ---- phase 3: attention, 16 query blocks ---------------------
            # Flat software pipeline over global key chunks u = ib*JQ + jq:
            #   iter t:  S-matmuls + exp of chunk t
            #            transposes + AT copy + O^T matmuls of chunk t-1
            #            epilogue of block (t-2)//JQ when t-2 ends a block
            # so the PE never sits on the exp (ACT) latency of its own chunk.
            o_sb = xo_pool.tile([P, KC, NH], DT, tag="xo")
            TOT = IB * JQ
            e_hold = {}
            ssum_hold = {}
            ot_hold = {}

            def stage_s(u):
                ib, jq = divmod(u, JQ)
                if jq == 0:
                    ssum_hold[ib] = small.tile([P, JQ], F32, tag="ssum", name=f"ssum{ib}")
                s_ps = ps.tile([P, 512], F32, tag="mm")
                for kc in range(KC):
                    nc.tensor.matmul(
                        s_ps[:], q_sb[:, kc, ib * P:(ib + 1) * P],
                        k_sb[:, kc, jq * 512:(jq + 1) * 512],
                        start=(kc == 0), stop=(kc == KC - 1))
                e_sb = epool.tile([P, 512], DT, tag="e")
                nc.scalar.activation(
                    e_sb[:], s_ps[:], mybir.ActivationFunctionType.Exp,
                    accum_out=ssum_hold[ib][:, jq:jq + 1])
                e_hold[u] = e_sb

            def stage_to(u):
                ib, jq = divmod(u, JQ)
                if jq == 0:
                    ot_hold[ib] = ps.tile([P, C], F32, tag="mm", name=f"ot{ib}")
                ot_ps = ot_hold[ib]
                e_sb = e_hold.pop(u)
                t_ps = ps.tile([P, 512], DT, tag="mm")
                for jj in range(4):
                    nc.tensor.transpose(
                        t_ps[:, jj * P:(jj + 1) * P],
                        e_sb[:, jj * P:(jj + 1) * P], ident_sb[:])
                at_sb = epool.tile([P, 4, P], DT, tag="at")
                tv = t_ps.rearrange("p (a b) -> p a b", b=P)
                nc.vector.tensor_copy(at_sb[:, 0:2], tv[:, 0:2])
                nc.vector.tensor_copy(at_sb[:, 2:4], tv[:, 2:4])
                for jj in range(4):
                    nc.tensor.matmul(
                        ot_ps[:], at_sb[:, jj, :], vt_sb[:, jq * 4 + jj, :],
                        start=(jq == 0 and jj == 0),
                        stop=(jq == JQ - 1 and jj == 3))

            def stage_epi(ib):
                ssum = ssum_hold.pop(ib)
                ot_ps = ot_hold.pop(ib)
                ssum_r = small.tile([P, 1], F32, tag="ssum_r")
                nc.vector.tensor_reduce(
                    ssum_r[:], ssum[:], mybir.AxisListType.X, mybir.AluOpType.add)
                recip = small.tile([P, 1], F32, tag="recip")
                nc.vector.reciprocal(recip[:], ssum_r[:])

                ot_sb = epool.tile([P, C], DT, tag="ot")
                nc.scalar.activation(
                    ot_sb[:], ot_ps[:], mybir.ActivationFunctionType.Copy,
                    scale=recip[:])
                to_ps = ps.tile([P, C], DT, tag="mm")
                for cb in range(KC):
                    nc.tensor.transpose(
                        to_ps[:, cb * P:(cb + 1) * P],
                        ot_sb[:, cb * P:(cb + 1) * P], ident_sb[:])
                nc.vector.tensor_copy(
                    o_sb[:, :, ib * P:(ib + 1) * P],
                    to_ps.rearrange("p (a b) -> p a b", b=P))

            # ---- phase 4 (interleaved): output projection + residual -----
            def stage_out(nt):
                for oc in range(KC):
                    pp = ps.tile([P, 512], F32, tag="mm")
                    for kc in range(KC):
                        nc.tensor.matmul(
                            pp[:], w_sb["wo"][:, kc, oc * P:(oc + 1) * P],
                            o_sb[:, kc, nt * 512:(nt + 1) * 512],
                            start=(kc == 0), stop=(kc == KC - 1))
                    x_res = rpool.tile([P, 512], F32, tag="xres")
                    nc.sync.dma_start(
                        x_res[:], xr_t[oc][:, nt * 512:(nt + 1) * 512])
                    r_sb = rpool.tile([P, 512], F32, tag="r")
                    nc.scalar.activation(
                        r_sb[:], pp[:], mybir.ActivationFunctionType.Identity,
                        bias=b_sb["bo"][:, oc:oc + 1], scale=1.0)
                    nc.vector.tensor_tensor(
                        r_sb[:], r_sb[:], x_res[:], mybir.AluOpType.add)
                    nc.sync.dma_start(y_t[oc][:, nt * 512:(nt + 1) * 512], r_sb[:])

            for t in range(TOT + 3):
                if t < TOT:
                    stage_s(t)
                if 1 <= t <= TOT:
                    stage_to(t - 1)
                if t >= 2 and (t - 2) % JQ == JQ - 1:
                    stage_epi((t - 2) // JQ)
                # wo-projection for 512-token slice nt once blocks
                # 4nt..4nt+3 have been epilogued (one chunk later).
                if t >= 3 and (t - 3) % (4 * JQ) == 4 * JQ - 1:
                    stage_out((t - 3) // (4 * JQ))

    return nc


def _prep_in_maps(inputs):
    x = np.asarray(inputs["x"], np.float32).reshape(4, C, N)
    s = np.float32(C ** -0.5)
    wq = np.asarray(inputs["wq"], np.float32)
    wk = np.asarray(inputs["wk"], np.float32)
    wv = np.asarray(inputs["wv"], np.float32)
    wo = np.asarray(inputs["wo"], np.float32)
    shared = {
        "wq": np.ascontiguousarray((wq * s).T).astype(np.float16),
        "wk": np.ascontiguousarray(wk.T).astype(np.float16),
        "wv": np.ascontiguousarray(wv.T).astype(np.float16),
        "wo": np.ascontiguousarray(wo.T).astype(np.float16),
        "bq": (np.asarray(inputs["bq"], np.float32) * s).astype(np.float32),
        "bk": np.asarray(inputs["bk"], np.float32),
        "bo": (wo @ np.asarray(inputs["bv"], np.float32)
               + np.asarray(inputs["bo"], np.float32)).astype(np.float32),
        "gam": np.asarray(inputs["gamma"], np.float32),
        "bet": np.asarray(inputs["beta"], np.float32),
        "gavg": (np.kron(np.eye(P // GS, dtype=np.float32),
                         np.ones((GS, GS), np.float32)) / GS),
        "ident": np.eye(P, dtype=np.float16),
    }
    in_maps = []
    for core in range(N_CORES):
        b, half = divmod(core, 2)
        xb = x[b]
        if half == 1:
            xrot = np.ascontiguousarray(
                np.concatenate([xb[:, NH:], xb[:, :NH]], axis=1))
        else:
            xrot = np.ascontiguousarray(xb)
        in_maps.append({"xr": xrot, **shared})
    return in_maps


def kernel_run(inputs, trace=False, trace_cores=None):
    """Run on all 8 cores; returns (full_output, BassKernelResults)."""
    from concourse.bass_utils import run_bass_kernel_spmd

    if "nc" not in _CACHE:
        _CACHE["nc"] = _build()
    nc = _CACHE["nc"]
    in_maps = _prep_in_maps(inputs)
    res = run_bass_kernel_spmd(
        nc, in_maps, core_ids=list(range(N_CORES)), trace=trace,
        trace_cores=trace_cores)
    out = np.empty((4, C, N), np.float32)
    for core in range(N_CORES):
        b, half = divmod(core, 2)
        out[b][:, half * NH:(half + 1) * NH] = res.results[core]["y"]
    return out.reshape(4, C, 64, 64), res


def kernel(**inputs):
    out, _ = kernel_run(inputs, trace=False)
    return out
